# revision 4
# baseline (speedup 1.0000x reference)
"""Trainium2 Bass kernel for the nms_detection competition problem.

For inputs plateau [2,256,256,32], phenotypes [2,128,32],
positions [2,128,2], alive [2,128,1]:

    masks   = relu(normalize(plateau_flat) @ normalize(phenotypes)^T)   [B,N,P]
    I       = (masks>.5)^T (masks>.5) over N  -> iou -> disputes -> alive'
    out     = masks * alive'^T

Sharding: 8 cores = 2 batches x 4 pixel shards (sequence-sharded over N).
Each core computes its [16384,128] mask slice in bf16 on the PE and streams
it straight to the output, accumulating binary-mask intersection partials
I_part [128,128] via PE matmuls.  The cross-shard reduction of the tiny
[P,P] partials and the O(P^2) compete logic happen on the host during the
gather/unshard step (the device->device collective for a 64KB payload costs
~35us of pure latency on this part, far more than the math it feeds).

Host-side prep per core (layout/sharding only): the bf16 cast of the
plateau slice in the two SBUF layouts the kernel wants (pixel-major for
row norms, (j,q)-major for the matmul lhsT) and the block-diagonal
normalized-phenotype operand.  Pixel norms are computed on device.

Device per-chunk dataflow (32 chunks of 512 pixels):
    PE   : pm[p,(j,pp)] = qT_c^T @ KD                (bf16, K=128 block-diag)
    DVE  : out_t = max(pm,0) * invq[(p,c,j)]         (fused relu+normalize)
    DMA  : out_t -> DRAM (optimistic, alive applied on host only if kills)
    DVE/Pool: mb = out_t > 0.5                       (bf16 binary masks)
    PE   : psI_j += mb_j^T @ mb_j                    (4 accumulating matmuls)
"""
import os
import numpy as np
import ml_dtypes

import concourse.bass as bass
import concourse.tile as tile
from concourse import mybir
from concourse import bass_utils
from contextlib import ExitStack

F32 = mybir.dt.float32
I16 = mybir.dt.int16
BF16 = mybir.dt.bfloat16

B, H, W, Q, P = 2, 256, 256, 32, 128
N = H * W                 # 65536 pixels per batch
NSHARD = 4                # pixel shards per batch
NCORE_PIX = N // NSHARD   # 16384 pixels per core
NCHUNK = 32               # chunks per core (512 pixels each)
N_CORES = 8
GROUPS = 8                # norm groups (4 chunks each)
GW = 512                  # columns per group in the [128, 4096] layouts

MASK_THRESH = 0.5
COMPETE_THRESH = 0.2
EPS = 1e-6

AluOp = mybir.AluOpType
ActFn = mybir.ActivationFunctionType


# ---------------------------------------------------------------------------
# Environment patches (walrus build here rejects >1 sync wait per instruction
# on the NO_STRUCT/S3_LW paths)
# ---------------------------------------------------------------------------
def _install_patches():
    if getattr(tile.TileContext, "_nms_drain_patched", False):
        return

    def _split_multiwaits(nc):
        """walrus here accepts at most one sync wait per instruction; move
        extra waits onto preceding same-engine NoOps."""
        ctr = [0]
        for bb in nc.main_func.blocks:
            insts = list(bb.instructions)
            if not any(i.sync_info is not None and len(i.sync_info.on_wait) > 1
                       for i in insts):
                continue
            new = []
            for inst in insts:
                si = inst.sync_info
                if si is not None and len(si.on_wait) > 1:
                    waits = list(si.on_wait)
                    for w in waits[:-1]:
                        ctr[0] += 1
                        nop = mybir.InstNoOp(
                            name=f"{inst.name}_wsplit{ctr[0]}",
                            engine=inst.engine,
                            bass_nofuse=True,
                            sync_info=mybir.SyncInfo(on_wait=[w], on_update=[]),
                        )
                        nc.register_instruction(nop, overwrite=True)
                        new.append(nop)
                    inst.sync_info = mybir.SyncInfo(
                        on_wait=[waits[-1]], on_update=list(si.on_update))
                new.append(inst)
            bb.instructions = new

    def _patched(self, tick_clock, wait_clock):
        from concourse.tile import ScopedClock
        drain_inst = self.nc.sync.drain()
        wait_clock.add_sem_waits(
            drain_inst.ins, ScopedClock({None: tick_clock.global_clock})
        )
        self.nc.all_engine_barrier()
        assert self.sems is not None
        popped = self.nc._tile_sem_poison_stack.pop()
        assert popped is self._sem_poison
        self.nc.clear_and_free_semaphores(list(self.sems.allocated().values()))
        self.nc.all_engine_barrier()
        _split_multiwaits(self.nc)

    tile.TileContext._drain_and_barrier = _patched
    tile.TileContext._nms_drain_patched = True

    # artifact upload would try to reach a share; keep everything local
    bass_utils.upload_artifacts = lambda tmpdir: tmpdir


_install_patches()


def _bcast_free(ap, reps):
    """AP view repeating each element of `ap` `reps` times along a new
    innermost free dim (step 0)."""
    return bass.AP(
        tensor=ap.tensor,
        offset=ap.offset,
        ap=list(ap.ap) + [[0, reps]],
    )


def build_kernel():
    nc = bass.Bass("TRN2", target_bir_lowering=False, debug=False,
                   enable_asserts=False, num_devices=N_CORES)

    # qt: lhsT layout [(j,q), (c,p)]; qn: norm layout [p, (c,j,q)]
    qt = nc.dram_tensor("qt", [128, NCHUNK * 128], BF16, kind="ExternalInput").ap()
    qn = nc.dram_tensor("qn", [128, NCHUNK * 128], BF16, kind="ExternalInput").ap()
    knt = nc.dram_tensor("knt", [Q, P], BF16, kind="ExternalInput").ap()
    out = nc.dram_tensor("out", [NCORE_PIX, P], BF16, kind="ExternalOutput").ap()
    ipart = nc.dram_tensor("ipart", [P, P], I16, kind="ExternalOutput").ap()

    # pixel n = 512c + 4p + j  <->  (chunk c, partition p, subrow j)
    outv = out.rearrange("(c p j) pp -> c p (j pp)", c=NCHUNK, p=128)

    with tile.TileContext(nc) as tc, ExitStack() as ctx:
        singles = ctx.enter_context(tc.tile_pool(name="singles", bufs=1))
        sqp = ctx.enter_context(tc.tile_pool(name="sqp", bufs=2))
        small = ctx.enter_context(tc.tile_pool(name="small", bufs=2))
        mpool = ctx.enter_context(tc.tile_pool(name="mpool", bufs=4))
        mbpool = ctx.enter_context(tc.tile_pool(name="mbpool", bufs=4))
        pmp = ctx.enter_context(tc.tile_pool(name="pmp", bufs=3, space="PSUM"))
        psacc = ctx.enter_context(tc.tile_pool(name="psacc", bufs=1, space="PSUM"))
        p2 = ctx.enter_context(tc.tile_pool(name="p2", bufs=1))

        v, sc, gp, te = nc.vector, nc.scalar, nc.gpsimd, nc.tensor

        # ------------------------------------------------------------------
        # inputs: KD block-diagonal phen operand + the two q layouts
        # ------------------------------------------------------------------
        KD = singles.tile([128, 512], BF16)
        v.memset(KD[:], 0.0)
        for j in range(4):
            nc.sync.dma_start(out=KD[32 * j:32 * (j + 1), 128 * j:128 * (j + 1)],
                              in_=knt)

        qt_s = singles.tile([128, NCHUNK * 128], BF16)
        qn_s = singles.tile([128, NCHUNK * 128], BF16)
        for g in range(GROUPS):
            e1 = nc.sync if g % 2 == 0 else nc.scalar
            e2 = nc.scalar if g % 2 == 0 else nc.sync
            e1.dma_start(out=qt_s[:, g * GW:(g + 1) * GW],
                         in_=qt[:, g * GW:(g + 1) * GW])
            e2.dma_start(out=qn_s[:, g * GW:(g + 1) * GW],
                         in_=qn[:, g * GW:(g + 1) * GW])

        # ------------------------------------------------------------------
        # pixel inverse norms, per group: invq[p, (c,j)]
        # ------------------------------------------------------------------
        invq = singles.tile([128, 128], F32)
        for g in range(GROUPS):
            sq = sqp.tile([128, GW], F32, tag="sq")
            sc.square(out=sq[:], in_=qn_s[:, g * GW:(g + 1) * GW])
            ss = small.tile([128, 16], F32, tag="ss")
            v.reduce_sum(out=ss[:],
                         in_=sq[:].rearrange("p (s q) -> p s q", q=Q),
                         axis=mybir.AxisListType.X)
            rt = small.tile([128, 16], F32, tag="rt")
            sc.sqrt(out=rt[:], in_=ss[:])
            v.tensor_scalar_max(out=rt[:], in0=rt[:], scalar1=EPS)
            v.reciprocal(out=invq[:, g * 16:(g + 1) * 16], in_=rt[:])

        # ------------------------------------------------------------------
        # main loop: masks, output stream, binary-mask intersections
        # ------------------------------------------------------------------
        psI = [psacc.tile([128, 128], F32, tag=f"psI{i}", name=f"psI{i}")
               for i in range(4)]
        for c in range(NCHUNK):
            pm = pmp.tile([128, 512], F32, tag="pm")
            te.matmul(out=pm[:], lhsT=qt_s[:, c * 128:(c + 1) * 128],
                      rhs=KD[:], start=True, stop=True)

            # fused relu + per-pixel normalize: max(pm,0) * invq[p, 4c+j]
            out_t = mpool.tile([128, 512], BF16, tag="m")
            v.scalar_tensor_tensor(
                out=out_t[:].rearrange("p (j f) -> p j f", j=4),
                in0=pm[:].rearrange("p (j f) -> p j f", j=4),
                scalar=0.0, op0=AluOp.max,
                in1=_bcast_free(invq[:, 4 * c:4 * (c + 1)], 128),
                op1=AluOp.mult,
            )
            # optimistic output write (masks without the alive filter);
            # the host zeroes killed columns only if someone dies (rare).
            (nc.sync if c % 2 == 0 else nc.scalar).dma_start(
                out=outv[c], in_=out_t[:])

            mb = mbpool.tile([128, 512], BF16, tag="mb")
            eng = v if c % 2 == 0 else gp
            eng.tensor_scalar(out=mb[:], in0=out_t[:], scalar1=MASK_THRESH,
                              scalar2=None, op0=AluOp.is_gt)

            for j in range(4):
                mbj = mb[:, 128 * j:128 * (j + 1)]
                te.matmul(out=psI[j][:], lhsT=mbj, rhs=mbj,
                          start=(c == 0), stop=(c == NCHUNK - 1),
                          skip_group_check=True)

        # ------------------------------------------------------------------
        # tail: fold the 4 PSUM partials, ship I_part (exact ints <= 16384)
        # ------------------------------------------------------------------
        Iacc = p2.tile([128, 128], F32, tag="Iacc")
        sc.copy(out=Iacc[:], in_=psI[0][:])
        for i in range(1, 4):
            v.tensor_tensor(out=Iacc[:], in0=Iacc[:], in1=psI[i][:],
                            op=AluOp.add)
        Icf = p2.tile([128, 128], I16, tag="Icf")
        v.tensor_copy(out=Icf[:], in_=Iacc[:])
        nc.sync.dma_start(out=ipart[:64, :], in_=Icf[:64, :])
        nc.scalar.dma_start(out=ipart[64:, :], in_=Icf[64:, :])

    return nc


_NC_CACHE = {}


def _get_nc():
    if "nc" not in _NC_CACHE:
        _NC_CACHE["nc"] = build_kernel()
    return _NC_CACHE["nc"]


def build_in_maps(plateau, phenotypes):
    """Host-side sharding/layout prep: bf16 casts of each core's plateau
    slice in the two layouts the kernel consumes, plus normalized-phenotype
    lhsT blocks."""
    q = np.ascontiguousarray(plateau, dtype=np.float32).reshape(B, N, Q)
    qb = q.astype(ml_dtypes.bfloat16)
    in_maps = []
    for b in range(B):
        ph = np.ascontiguousarray(phenotypes[b], dtype=np.float32)
        kn = ph / np.maximum(np.linalg.norm(ph, axis=-1, keepdims=True), EPS)
        knt = np.ascontiguousarray(kn.astype(ml_dtypes.bfloat16).T)  # [Q, P]
        for s in range(NSHARD):
            sl = qb[b, s * NCORE_PIX:(s + 1) * NCORE_PIX]  # [16384, 32]
            four = sl.reshape(NCHUNK, 128, 4, Q)           # [c, p, j, q]
            qn_pre = np.ascontiguousarray(
                four.transpose(1, 0, 2, 3)).reshape(128, NCHUNK * 128)
            qt_pre = np.ascontiguousarray(
                four.transpose(2, 3, 0, 1)).reshape(128, NCHUNK * 128)
            in_maps.append({"qt": qt_pre, "qn": qn_pre, "knt": knt})
    return in_maps


def _fitness(plateau, phenotypes, positions):
    """numpy port of the reference bilinear-gather compatibility, [B,P]."""
    h = (positions[..., 0] + 1.0) * H * 0.5
    w = (positions[..., 1] + 1.0) * W * 0.5
    h = np.clip(h, 0.0, H - 1)
    w = np.clip(w, 0.0, W - 1)
    hf, wf = np.floor(h), np.floor(w)
    hc, wc = np.ceil(h), np.ceil(w)
    br = (h - hf) * (w - wf)
    bl = (h - hf) * (wc - w)
    tr = (hc - h) * (w - wf)
    tl = (hc - h) * (wc - w)
    ib = np.arange(B)[:, None]

    def g(hi, wi):
        return plateau[ib, hi.astype(np.int32), wi.astype(np.int32)]  # [B,P,Q]

    pv = (g(hf, wf) * tl[..., None] + g(hf, wc) * tr[..., None]
          + g(hc, wf) * bl[..., None] + g(hc, wc) * br[..., None])
    pv = pv / np.maximum(np.linalg.norm(pv, axis=-1, keepdims=True), EPS)
    kn = phenotypes / np.maximum(
        np.linalg.norm(phenotypes, axis=-1, keepdims=True), EPS)
    return np.sum(kn * pv, axis=-1)  # [B,P]


def _compete(I, fit, alive):
    """numpy port of the reference compete logic. I [B,P,P] exact counts."""
    s = np.einsum('bpp->bp', I)  # diag(I) = mask areas
    U = s[:, :, None] + s[:, None, :] - I
    iou = I / np.maximum(U, EPS)
    eye = np.eye(P, dtype=bool)[None]
    disputes = (iou > COMPETE_THRESH) & ~eye
    killed = disputes & (fit[:, :, None] < fit[:, None, :])
    winners = alive[..., 0] > 0.5
    losers = ~winners
    killed = killed & ~(winners[:, :, None] & losers[:, None, :])
    killed = killed | ((losers[:, :, None] & winners[:, None, :]) & disputes)
    return (~killed.any(axis=2)).astype(np.float32)  # [B,P]


def postprocess(results, plateau, phenotypes, positions, alive):
    """Gather/unshard: assemble the full mask tensor, reduce the tiny
    [P,P] intersection partials across shards, run compete, and apply the
    alive filter (a 0/1 column mask) if any agent was killed."""
    out = np.empty((B, N, P), dtype=np.float32)
    I = np.zeros((B, P, P), dtype=np.float64)
    for b in range(B):
        for s in range(NSHARD):
            r = results[b * NSHARD + s]
            out[b, s * NCORE_PIX:(s + 1) * NCORE_PIX] = \
                np.asarray(r["out"]).astype(np.float32)
            I[b] += np.asarray(r["ipart"]).astype(np.float64)
    fit = _fitness(np.ascontiguousarray(plateau, np.float32),
                   np.ascontiguousarray(phenotypes, np.float32),
                   np.ascontiguousarray(positions, np.float32))
    alive_new = _compete(I, fit, np.ascontiguousarray(alive, np.float32))
    if (alive_new < 0.5).any():
        out *= alive_new[:, None, :]
    return out


def kernel(plateau, phenotypes, positions, alive):
    nc = _get_nc()
    in_maps = build_in_maps(plateau, phenotypes)
    res = bass_utils.run_bass_kernel_spmd(
        nc, in_maps, core_ids=list(range(N_CORES)))
    return postprocess(res.results, plateau, phenotypes, positions, alive)


# revision 12
# speedup vs baseline: 2.8245x; 2.8245x over previous
"""Trainium2 Bass kernel for the nms_detection competition problem.

For inputs plateau [2,256,256,32], phenotypes [2,128,32],
positions [2,128,2], alive [2,128,1]:

    masks   = relu(normalize(plateau_flat) @ normalize(phenotypes)^T)   [B,N,P]
    I       = (masks>.5)^T (masks>.5) over N  -> iou -> disputes -> alive'
    out     = masks * alive'^T

Sharding: 8 cores = 2 batches x 4 pixel shards (sequence-sharded over N).
Each core computes its [16384,128] mask slice in bf16 on the PE and streams
it straight to the output, accumulating binary-mask intersection partials
I_part [128,128] via PE matmuls.  The cross-shard reduction of the tiny
[P,P] partials and the O(P^2) compete logic happen on the host during the
gather/unshard step (the device->device collective for a 64KB payload costs
~35us of pure latency on this part, far more than the math it feeds).

Host-side prep per core (layout/sharding only): the bf16 cast of the
plateau slice in the two SBUF layouts the kernel wants (pixel-major for
row norms, (j,q)-major for the matmul lhsT) and the block-diagonal
normalized-phenotype operand.  Pixel norms are computed on device.

Device per-chunk dataflow (32 chunks of 512 pixels):
    PE   : pm[p,(j,pp)] = qT_c^T @ KD                (bf16, K=128 block-diag)
    DVE  : out_t = max(pm,0) * invq[(p,c,j)]         (fused relu+normalize)
    DMA  : out_t -> DRAM (optimistic, alive applied on host only if kills)
    DVE/Pool: mb = out_t > 0.5                       (bf16 binary masks)
    PE   : psI_j += mb_j^T @ mb_j                    (4 accumulating matmuls)
"""
import os
import numpy as np
import ml_dtypes

import concourse.bass as bass
import concourse.tile as tile
from concourse import mybir
from concourse import bass_utils
from contextlib import ExitStack

F32 = mybir.dt.float32
I16 = mybir.dt.int16
BF16 = mybir.dt.bfloat16

B, H, W, Q, P = 2, 256, 256, 32, 128
N = H * W                 # 65536 pixels per batch
NSHARD = 4                # pixel shards per batch
NCORE_PIX = N // NSHARD   # 16384 pixels per core
NCHUNK = 32               # chunks per core (512 pixels each)
N_CORES = 8
GROUPS = 8                # norm groups (4 chunks each)
GW = 512                  # columns per group in the [128, 4096] layouts

MASK_THRESH = 0.5
COMPETE_THRESH = 0.2
EPS = 1e-6

AluOp = mybir.AluOpType
ActFn = mybir.ActivationFunctionType


# ---------------------------------------------------------------------------
# Environment patches (walrus build here rejects >1 sync wait per instruction
# on the NO_STRUCT/S3_LW paths)
# ---------------------------------------------------------------------------
def _install_patches():
    if getattr(tile.TileContext, "_nms_drain_patched", False):
        return

    def _split_multiwaits(nc):
        """walrus here accepts at most one sync wait per instruction; move
        extra waits onto preceding same-engine NoOps."""
        ctr = [0]
        for bb in nc.main_func.blocks:
            insts = list(bb.instructions)
            if not any(i.sync_info is not None and len(i.sync_info.on_wait) > 1
                       for i in insts):
                continue
            new = []
            for inst in insts:
                si = inst.sync_info
                if si is not None and len(si.on_wait) > 1:
                    waits = list(si.on_wait)
                    for w in waits[:-1]:
                        ctr[0] += 1
                        nop = mybir.InstNoOp(
                            name=f"{inst.name}_wsplit{ctr[0]}",
                            engine=inst.engine,
                            bass_nofuse=True,
                            sync_info=mybir.SyncInfo(on_wait=[w], on_update=[]),
                        )
                        nc.register_instruction(nop, overwrite=True)
                        new.append(nop)
                    inst.sync_info = mybir.SyncInfo(
                        on_wait=[waits[-1]], on_update=list(si.on_update))
                new.append(inst)
            bb.instructions = new

    def _patched(self, tick_clock, wait_clock):
        from concourse.tile import ScopedClock
        drain_inst = self.nc.sync.drain()
        wait_clock.add_sem_waits(
            drain_inst.ins, ScopedClock({None: tick_clock.global_clock})
        )
        self.nc.all_engine_barrier()
        assert self.sems is not None
        popped = self.nc._tile_sem_poison_stack.pop()
        assert popped is self._sem_poison
        self.nc.clear_and_free_semaphores(list(self.sems.allocated().values()))
        self.nc.all_engine_barrier()
        _split_multiwaits(self.nc)

    tile.TileContext._drain_and_barrier = _patched
    tile.TileContext._nms_drain_patched = True

    # artifact upload would try to reach a share; keep everything local
    bass_utils.upload_artifacts = lambda tmpdir: tmpdir


_install_patches()


def _bcast_free(ap, reps):
    """AP view repeating each element of `ap` `reps` times along a new
    innermost free dim (step 0)."""
    return bass.AP(
        tensor=ap.tensor,
        offset=ap.offset,
        ap=list(ap.ap) + [[0, reps]],
    )


def build_kernel():
    nc = bass.Bass("TRN2", target_bir_lowering=False, debug=False,
                   enable_asserts=False, num_devices=N_CORES)

    # qt: lhsT layout [(j,q), (c,p)]; qn: norm layout [p, (c,j,q)]
    qt = nc.dram_tensor("qt", [128, NCHUNK * 128], BF16, kind="ExternalInput").ap()
    qn = nc.dram_tensor("qn", [128, NCHUNK * 128], BF16, kind="ExternalInput").ap()
    knt = nc.dram_tensor("knt", [Q, P], BF16, kind="ExternalInput").ap()
    out = nc.dram_tensor("out", [NCORE_PIX, P], BF16, kind="ExternalOutput").ap()
    ipart = nc.dram_tensor("ipart", [P, P], I16, kind="ExternalOutput").ap()

    # pixel n = 512c + 4p + j  <->  (chunk c, partition p, subrow j)
    outv = out.rearrange("(c p j) pp -> c p (j pp)", c=NCHUNK, p=128)

    with tile.TileContext(nc) as tc, ExitStack() as ctx:
        singles = ctx.enter_context(tc.tile_pool(name="singles", bufs=1))
        sqp = ctx.enter_context(tc.tile_pool(name="sqp", bufs=2))
        small = ctx.enter_context(tc.tile_pool(name="small", bufs=2))
        mpool = ctx.enter_context(tc.tile_pool(name="mpool", bufs=4))
        mbpool = ctx.enter_context(tc.tile_pool(name="mbpool", bufs=4))
        pmp = ctx.enter_context(tc.tile_pool(name="pmp", bufs=3, space="PSUM"))
        psacc = ctx.enter_context(tc.tile_pool(name="psacc", bufs=1, space="PSUM"))
        p2 = ctx.enter_context(tc.tile_pool(name="p2", bufs=1))

        v, sc, gp, te = nc.vector, nc.scalar, nc.gpsimd, nc.tensor

        # ------------------------------------------------------------------
        # inputs.  All HWDGE triggers cost ~600ns of engine time, so the
        # sync engine owns every input trigger; output DMAs go through the
        # gpsimd software DGE whose triggers are cheap.
        # ------------------------------------------------------------------
        KD = singles.tile([128, 512], BF16)
        gp.memset(KD[:], 0.0)
        for j in range(4):
            nc.scalar.dma_start(out=KD[32 * j:32 * (j + 1), 128 * j:128 * (j + 1)],
                                in_=knt)

        # preload the activation table (square/rsqrt/relu share one table)
        # before the first real activate, concurrent with the input DMAs
        dumm = small.tile([128, 2], F32, tag="dumm")
        gp.memset(dumm[:], 1.0)
        dumm2 = small.tile([128, 2], F32, tag="dumm2")
        sc.square(out=dumm2[:], in_=dumm[:])

        qt_s = singles.tile([128, NCHUNK * 128], BF16)
        qn_s = singles.tile([128, NCHUNK * 128], BF16)
        for g in range(GROUPS):
            nc.sync.dma_start(out=qt_s[:, g * GW:(g + 1) * GW],
                              in_=qt[:, g * GW:(g + 1) * GW])
            nc.sync.dma_start(out=qn_s[:, g * GW:(g + 1) * GW],
                              in_=qn[:, g * GW:(g + 1) * GW])

        # ------------------------------------------------------------------
        # pixel inverse norms, per group: invq[p, (c,j)] = rsqrt(sumsq)
        # (fp32 copy for the Act-engine scale operand, bf16 for the DVE path)
        # ------------------------------------------------------------------
        invq = singles.tile([128, 128], F32)
        invqb = singles.tile([128, 128], BF16)
        ss_all = singles.tile([128, 128], F32)
        for g in range(GROUPS):
            sq = sqp.tile([128, GW], F32, tag="sq")
            sc.square(out=sq[:], in_=qn_s[:, g * GW:(g + 1) * GW])
            v.reduce_sum(out=ss_all[:, g * 16:(g + 1) * 16],
                         in_=sq[:].rearrange("p (s q) -> p s q", q=Q),
                         axis=mybir.AxisListType.X)
        # batched over 4 groups: 1/max(sqrt(ss),eps) == 1/sqrt(ss+eps^2)
        # up to degenerate rows
        for h in range(2):
            rt = small.tile([128, 64], F32, tag="rt")
            sc.sqrt(out=rt[:], in_=ss_all[:, 64 * h:64 * (h + 1)])
            v.tensor_scalar_max(out=rt[:], in0=rt[:], scalar1=EPS)
            v.reciprocal(out=invq[:, 64 * h:64 * (h + 1)], in_=rt[:])
            v.tensor_copy(out=invqb[:, 64 * h:64 * (h + 1)],
                          in_=invq[:, 64 * h:64 * (h + 1)])

        # ------------------------------------------------------------------
        # main loop: masks, output stream, binary-mask intersections.
        # Chunks 0..27 write output in 256KB pair-DMAs (halves the ~600ns
        # HWDGE triggers); the last 4 go as singles so the final drain is
        # short.
        # ------------------------------------------------------------------
        NPAIRED = 28

        def pair_dst(d):
            # dst AP for chunks (2d, 2d+1): dims (p, e, j, pp) matching the
            # [128, (e j pp)] pair tile; strides in elements of `out`.
            return bass.AP(tensor=out.tensor,
                           offset=out.offset + d * 1024 * P,
                           ap=[[4 * P, 128], [512 * P, 2], [P, 4], [1, P]])
        psI = [psacc.tile([128, 128], F32, tag=f"psI{i}", name=f"psI{i}")
               for i in range(4)]
        pt = None
        for c in range(NCHUNK):
            pm = pmp.tile([128, 512], F32, tag="pm")
            te.matmul(out=pm[:], lhsT=qt_s[:, c * 128:(c + 1) * 128],
                      rhs=KD[:], start=True, stop=True)

            if c >= NPAIRED:
                pt = mpool.tile([128, 512], BF16, tag="ms")
                base = 0
            elif c % 2 == 0:
                pt = mpool.tile([128, 1024], BF16, tag="mp")
                base = 0
            else:
                base = 512

            # fused relu + per-pixel normalize: max(pm,0) * invq[p, 4c+j]
            if c % 2 == 0:
                v.scalar_tensor_tensor(
                    out=pt[:, base:base + 512].rearrange(
                        "p (j f) -> p j f", j=4),
                    in0=pm[:].rearrange("p (j f) -> p j f", j=4),
                    scalar=0.0, op0=AluOp.max,
                    in1=_bcast_free(invqb[:, 4 * c:4 * (c + 1)], 128),
                    op1=AluOp.mult,
                )
            else:
                for j in range(4):
                    col = 4 * c + j
                    sc.activation(out=pt[:, base + 128 * j:base + 128 * (j + 1)],
                                  in_=pm[:, 128 * j:128 * (j + 1)],
                                  func=ActFn.Relu,
                                  scale=invq[:, col:col + 1])
            # optimistic output write (masks without the alive filter);
            # the host zeroes killed columns only if someone dies (rare).
            if c >= NPAIRED:
                (nc.sync if c % 2 == 0 else nc.scalar).dma_start(
                    out=outv[c], in_=pt[:])
            elif c % 2 == 1:
                d = c // 2
                (nc.sync if d % 2 == 0 else nc.scalar).dma_start(
                    out=pair_dst(d), in_=pt[:])

            mb = mbpool.tile([128, 512], BF16, tag="mb")
            v.tensor_scalar(out=mb[:], in0=pt[:, base:base + 512],
                            scalar1=MASK_THRESH, scalar2=None,
                            op0=AluOp.is_gt)

            for j in range(4):
                mbj = mb[:, 128 * j:128 * (j + 1)]
                te.matmul(out=psI[j][:], lhsT=mbj, rhs=mbj,
                          start=(c == 0), stop=(c == NCHUNK - 1),
                          skip_group_check=True)

        # ------------------------------------------------------------------
        # tail: fold the 4 PSUM partials, ship I_part (exact ints <= 16384)
        # ------------------------------------------------------------------
        Iacc = p2.tile([128, 128], F32, tag="Iacc")
        sc.copy(out=Iacc[:], in_=psI[0][:])
        for i in range(1, 4):
            v.tensor_tensor(out=Iacc[:], in0=Iacc[:], in1=psI[i][:],
                            op=AluOp.add)
        Icf = p2.tile([128, 128], I16, tag="Icf")
        v.tensor_copy(out=Icf[:], in_=Iacc[:])
        nc.sync.dma_start(out=ipart[:64, :], in_=Icf[:64, :])
        nc.scalar.dma_start(out=ipart[64:, :], in_=Icf[64:, :])

    return nc


_NC_CACHE = {}


def _get_nc():
    if "nc" not in _NC_CACHE:
        _NC_CACHE["nc"] = build_kernel()
    return _NC_CACHE["nc"]


def build_in_maps(plateau, phenotypes):
    """Host-side sharding/layout prep: bf16 casts of each core's plateau
    slice in the two layouts the kernel consumes, plus normalized-phenotype
    lhsT blocks."""
    q = np.ascontiguousarray(plateau, dtype=np.float32).reshape(B, N, Q)
    qb = q.astype(ml_dtypes.bfloat16)
    in_maps = []
    for b in range(B):
        ph = np.ascontiguousarray(phenotypes[b], dtype=np.float32)
        kn = ph / np.maximum(np.linalg.norm(ph, axis=-1, keepdims=True), EPS)
        knt = np.ascontiguousarray(kn.astype(ml_dtypes.bfloat16).T)  # [Q, P]
        for s in range(NSHARD):
            sl = qb[b, s * NCORE_PIX:(s + 1) * NCORE_PIX]  # [16384, 32]
            four = sl.reshape(NCHUNK, 128, 4, Q)           # [c, p, j, q]
            qn_pre = np.ascontiguousarray(
                four.transpose(1, 0, 2, 3)).reshape(128, NCHUNK * 128)
            qt_pre = np.ascontiguousarray(
                four.transpose(2, 3, 0, 1)).reshape(128, NCHUNK * 128)
            in_maps.append({"qt": qt_pre, "qn": qn_pre, "knt": knt})
    return in_maps


def _fitness(plateau, phenotypes, positions):
    """numpy port of the reference bilinear-gather compatibility, [B,P]."""
    h = (positions[..., 0] + 1.0) * H * 0.5
    w = (positions[..., 1] + 1.0) * W * 0.5
    h = np.clip(h, 0.0, H - 1)
    w = np.clip(w, 0.0, W - 1)
    hf, wf = np.floor(h), np.floor(w)
    hc, wc = np.ceil(h), np.ceil(w)
    br = (h - hf) * (w - wf)
    bl = (h - hf) * (wc - w)
    tr = (hc - h) * (w - wf)
    tl = (hc - h) * (wc - w)
    ib = np.arange(B)[:, None]

    def g(hi, wi):
        return plateau[ib, hi.astype(np.int32), wi.astype(np.int32)]  # [B,P,Q]

    pv = (g(hf, wf) * tl[..., None] + g(hf, wc) * tr[..., None]
          + g(hc, wf) * bl[..., None] + g(hc, wc) * br[..., None])
    pv = pv / np.maximum(np.linalg.norm(pv, axis=-1, keepdims=True), EPS)
    kn = phenotypes / np.maximum(
        np.linalg.norm(phenotypes, axis=-1, keepdims=True), EPS)
    return np.sum(kn * pv, axis=-1)  # [B,P]


def _compete(I, fit, alive):
    """numpy port of the reference compete logic. I [B,P,P] exact counts."""
    s = np.einsum('bpp->bp', I)  # diag(I) = mask areas
    U = s[:, :, None] + s[:, None, :] - I
    iou = I / np.maximum(U, EPS)
    eye = np.eye(P, dtype=bool)[None]
    disputes = (iou > COMPETE_THRESH) & ~eye
    killed = disputes & (fit[:, :, None] < fit[:, None, :])
    winners = alive[..., 0] > 0.5
    losers = ~winners
    killed = killed & ~(winners[:, :, None] & losers[:, None, :])
    killed = killed | ((losers[:, :, None] & winners[:, None, :]) & disputes)
    return (~killed.any(axis=2)).astype(np.float32)  # [B,P]


def postprocess(results, plateau, phenotypes, positions, alive):
    """Gather/unshard: assemble the full mask tensor, reduce the tiny
    [P,P] intersection partials across shards, run compete, and apply the
    alive filter (a 0/1 column mask) if any agent was killed."""
    out = np.empty((B, N, P), dtype=np.float32)
    I = np.zeros((B, P, P), dtype=np.float64)
    for b in range(B):
        for s in range(NSHARD):
            r = results[b * NSHARD + s]
            out[b, s * NCORE_PIX:(s + 1) * NCORE_PIX] = \
                np.asarray(r["out"]).astype(np.float32)
            I[b] += np.asarray(r["ipart"]).astype(np.float64)
    fit = _fitness(np.ascontiguousarray(plateau, np.float32),
                   np.ascontiguousarray(phenotypes, np.float32),
                   np.ascontiguousarray(positions, np.float32))
    alive_new = _compete(I, fit, np.ascontiguousarray(alive, np.float32))
    if (alive_new < 0.5).any():
        out *= alive_new[:, None, :]
    return out


def kernel(plateau, phenotypes, positions, alive):
    nc = _get_nc()
    in_maps = build_in_maps(plateau, phenotypes)
    res = bass_utils.run_bass_kernel_spmd(
        nc, in_maps, core_ids=list(range(N_CORES)))
    return postprocess(res.results, plateau, phenotypes, positions, alive)
